# revision 12
# baseline (speedup 1.0000x reference)
"""CoedgeConvLayer Trainium2 kernel.

y = relu(x @ W_self + x[next] @ W_next + x[prev] @ W_prev + x[mate] @ W_mate + b_tot)

Sharding: rows (coedges) data-parallel across 8 NeuronCores; the full feature
table is replicated per core so neighbor gathers are purely local (no
collectives).  The SPMD program is identical on every core — all per-core
differences live in the index data: the self stream is expressed as a gather
with indices base + row, so one batched indirect DMA per block fetches all 4
streams (4*G*128 rows), amortizing the SWDGE fixed cost.  Each 128-row subtile
is transposed on the tensor engine (via identity matmul) so the contraction
dim lands on partitions, then 8 accumulating matmuls (4 streams x 2 K-chunks)
plus a K=1 bias outer product run into PSUM, and a fused ReLU copy moves the
result to SBUF for the block store.
"""

import numpy as np

import concourse.bass as bass
from concourse import bacc
import concourse.mybir as mybir
import concourse.tile as tile
from concourse import bass_utils
from concourse.masks import make_identity

# Problem constants (hardcoded per harness contract).
N = 200000
D = 256
NCORES = 8
ROWS_PER_CORE = N // NCORES          # 25000
P = 128
SUBTILES = (ROWS_PER_CORE + P - 1) // P   # 196
PAD_ROWS = SUBTILES * P              # 25088
G = 7                                # subtiles per block
NBLOCKS = SUBTILES // G              # 28
KCHUNKS = 2                          # 256 = 2 * 128
NSTREAMS = 4                         # self, next, prev, mate

# Compute dtype for features/weights on device. float32 is exact; bfloat16
# halves gather traffic (this kernel is memory-bound) at ~1e-3 rel err.
import os
USE_BF16 = os.environ.get("KERNEL_BF16", "0") == "1"

if USE_BF16:
    import ml_dtypes
    _FEAT_DT = mybir.dt.bfloat16
    _FEAT_NP = ml_dtypes.bfloat16
else:
    _FEAT_DT = mybir.dt.float32
    _FEAT_NP = np.float32


def _build_nc(repeat=1):
    nc = bacc.Bacc("TRN2", debug=False, enable_partition_id=False)
    f32 = mybir.dt.float32
    feats = nc.dram_tensor("features", [N, D], _FEAT_DT, kind="ExternalInput")
    w = nc.dram_tensor("w", [NSTREAMS * D, D], _FEAT_DT, kind="ExternalInput")
    bias = nc.dram_tensor("bias", [1, D], _FEAT_DT, kind="ExternalInput")
    idx = nc.dram_tensor("idx", [P, NBLOCKS * NSTREAMS * G], mybir.dt.int32,
                         kind="ExternalInput")
    out = nc.dram_tensor("out", [PAD_ROWS, D], f32, kind="ExternalOutput")

    feats_ap = feats.ap()
    out_ap = out.ap()
    SG = NSTREAMS * G                # index / gather columns per block

    with tile.TileContext(nc) as tc:
        with (
            tc.tile_pool(name="const", bufs=1) as cpool,
            tc.tile_pool(name="gather", bufs=12) as gpool,
            tc.tile_pool(name="xt", bufs=6) as xtpool,
            tc.tile_pool(name="outp", bufs=2) as opool,
            tc.tile_pool(name="pt", bufs=4, space="PSUM") as ptpool,
            tc.tile_pool(name="pacc", bufs=3, space="PSUM") as paccpool,
        ):
            # Resident constants.
            w_sb = cpool.tile([P, NSTREAMS * KCHUNKS, D], _FEAT_DT)
            nc.sync.dma_start(
                out=w_sb[:], in_=w.ap().rearrange("(c p) n -> p c n", p=P))
            bias_sb = cpool.tile([1, D], _FEAT_DT)
            nc.sync.dma_start(out=bias_sb[:], in_=bias.ap())
            idx_sb = cpool.tile([P, NBLOCKS * SG], mybir.dt.int32)
            nc.sync.dma_start(out=idx_sb[:], in_=idx.ap())
            ident = cpool.tile([P, P], _FEAT_DT)
            make_identity(nc, ident[:])
            ones_sb = cpool.tile([1, P], _FEAT_DT)
            nc.gpsimd.memset(ones_sb[:], 1.0)
            # Priming transpose: folds the gpsimd-preamble wait into PE's
            # vector clock so steady-state PE instructions need at most one
            # sem wait (the lowered LDWEIGHTS struct has a single wait slot).
            pt0 = ptpool.tile([P, P], _FEAT_DT, tag='pt')
            nc.tensor.transpose(pt0[:], ident[:], ident[:])

            for b in range(NBLOCKS * repeat):
                b = b % NBLOCKS
                r0 = b * G * P
                outsb = opool.tile([P, G, D], mybir.dt.float32)
                for g in range(G):
                    # Gather the 4 stream subtiles (128 rows each).  The HW
                    # SWDGE path only supports one offset per partition, so
                    # each gather is a separate [P,1]-offset instruction.
                    srcs = []
                    for s in range(NSTREAMS):
                        col = b * SG + s * G + g
                        xgt = gpool.tile([P, D], _FEAT_DT, tag="xg")
                        nc.gpsimd.indirect_dma_start(
                            out=xgt[:],
                            out_offset=None,
                            in_=feats_ap,
                            in_offset=bass.IndirectOffsetOnAxis(
                                ap=idx_sb[:, col:col + 1], axis=0),
                        )
                        srcs.append(xgt)
                    # Transpose the 4 stream subtiles so d_in is on partitions.
                    xts = []
                    for s in range(NSTREAMS):
                        src = srcs[s][:]
                        for ki in range(KCHUNKS):
                            pt = ptpool.tile([P, P], _FEAT_DT, tag='pt')
                            nc.tensor.transpose(
                                pt[:], src[:, ki * P:(ki + 1) * P], ident[:])
                            xt = xtpool.tile([P, P], _FEAT_DT)
                            nc.vector.tensor_copy(out=xt[:], in_=pt[:])
                            xts.append(xt)
                    pacc = paccpool.tile([P, D], mybir.dt.float32)
                    for c, xt in enumerate(xts):
                        nc.tensor.matmul(
                            pacc[:], lhsT=xt[:], rhs=w_sb[:, c, :],
                            start=(c == 0), stop=False)
                    # Bias as a K=1 outer product: ones[128] x b_tot[256].
                    nc.tensor.matmul(
                        pacc[:], lhsT=ones_sb[:1, :], rhs=bias_sb[:1, :],
                        start=False, stop=True)
                    # Fused ReLU on the PSUM -> SBUF move.
                    nc.scalar.activation(
                        outsb[:, g, :], pacc[:],
                        mybir.ActivationFunctionType.Relu)
                nc.sync.dma_start(
                    out=out_ap[r0:r0 + G * P, :].rearrange(
                        "(g p) n -> p g n", p=P),
                    in_=outsb[:],
                )
    nc.compile()
    return nc


def _prepare_in_maps(features, next_indices, prev_indices, mate_indices,
                     W_self, b_self, W_next, b_next, W_prev, b_prev,
                     W_mate, b_mate):
    feats = np.ascontiguousarray(
        np.asarray(features, dtype=np.float32).astype(_FEAT_NP))

    w_cat = np.concatenate(
        [np.asarray(W_self, np.float32), np.asarray(W_next, np.float32),
         np.asarray(W_prev, np.float32), np.asarray(W_mate, np.float32)],
        axis=0).astype(_FEAT_NP)
    w_cat = np.ascontiguousarray(w_cat)
    b_tot = (np.asarray(b_self, np.float32) + np.asarray(b_next, np.float32)
             + np.asarray(b_prev, np.float32) + np.asarray(b_mate, np.float32))
    b_tot = np.ascontiguousarray(b_tot.reshape(1, D).astype(_FEAT_NP))

    nbr = [np.asarray(next_indices), np.asarray(prev_indices),
           np.asarray(mate_indices)]

    in_maps = []
    for c in range(NCORES):
        base = c * ROWS_PER_CORE
        # idx layout: [P, NBLOCKS, NSTREAMS, G]; value = stream index of local
        # row (b*G + g)*128 + p (0 for pad rows, which are discarded).
        idx_arr = np.zeros((P, NBLOCKS, NSTREAMS, G), dtype=np.int32)
        loc = np.zeros(PAD_ROWS, dtype=np.int64)
        loc[:ROWS_PER_CORE] = base + np.arange(ROWS_PER_CORE, dtype=np.int64)
        idx_arr[:, :, 0, :] = (
            loc.reshape(NBLOCKS, G, P).transpose(2, 0, 1).astype(np.int32))
        for s, I in enumerate(nbr):
            loc = np.zeros(PAD_ROWS, dtype=np.int64)
            loc[:ROWS_PER_CORE] = I[base:base + ROWS_PER_CORE]
            idx_arr[:, :, s + 1, :] = (
                loc.reshape(NBLOCKS, G, P).transpose(2, 0, 1).astype(np.int32))
        idx_flat = np.ascontiguousarray(
            idx_arr.reshape(P, NBLOCKS * NSTREAMS * G))
        in_maps.append({
            "features": feats,
            "w": w_cat,
            "bias": b_tot,
            "idx": idx_flat,
        })
    return in_maps


def kernel(**inputs) -> np.ndarray:
    in_maps = _prepare_in_maps(**inputs)
    nc = _build_nc()
    res = bass_utils.run_bass_kernel_spmd(
        nc, in_maps, core_ids=list(range(NCORES)))
    out = np.concatenate(
        [res.results[c]["out"][:ROWS_PER_CORE] for c in range(NCORES)], axis=0)
    return np.ascontiguousarray(out.astype(np.float32))
